# revision 14
# baseline (speedup 1.0000x reference)
"""DCGRU cell on 8 Trainium2 NeuronCores (Bass/Tile), v3.

Math: with a = adj + I, d = a.sum(axis=1), T = (d^-1 a)^T, every
diffusion step is  y = T @ v = a^T @ (d_inv * v).  d_inv is computed on
the HOST, so there are no row-sum collectives.  The d_inv factor rides
on the activation side: the stationary operand of each diffusion matmul
is z = c * d_inv * v (c a power of 2 keeping fp8 values in normal
range).  All unscale constants fold into the host-prepared gate weights
(W0' = W0 - W2, W1' = W1/c0, W2' = 2*W2/c1), so diffusion PSUMs are
evacuated as raw bf16 copies and the Chebyshev combine x2 = 2*T@x1 - x0
never materializes.

Sharding (8 cores): 1D column-parallel over the adjacency.  Core m
holds a[:, m*1024:(m+1)*1024] as fp8e4 (8 MB), host-permuted p-major so
partition lines are contiguous.  Each diffusion is a DoubleRow fp8
matmul: stationary z pair-chunk [128, 2, 128], moving adjacency
[128, 2, 512] -> psum [128, 512]; output is feature-major [b*32+j, n]
which is what the gate matmuls consume directly.  AllGather payloads
are node-major padded fp8, pre-scaled by c*d_inv during the
transpose-evacuation, so the gather DMA writes the next diffusion's
stationary operand directly (no on-device z build at all).

Feature order is h-first: j = 0..15 hidden, j = 16,17 input; padded to
32 rows/cols per batch so partition bases stay 0/32/64/96.
"""

import numpy as np
import ml_dtypes

import concourse.bass as bass
import concourse.bacc as bacc
import concourse.tile as tile
import concourse.mybir as mybir
import concourse.bass_utils as bass_utils

F32 = mybir.dt.float32
BF16 = mybir.dt.bfloat16
FP8 = mybir.dt.float8e4
AF = mybir.ActivationFunctionType
ALU = mybir.AluOpType
DR = mybir.MatmulPerfMode.DoubleRow

NCORES = 8
N = 8192          # nodes
C = N // NCORES   # own nodes per core (1024)
P = 128           # partitions
KCH = N // P      # node chunks (64)
KP = KCH // 2     # DoubleRow chunk pairs (32)
MB = C // P       # own-node tiles (8)
B = 4             # batch
IT = 18           # I_tot = in_dim + units
FP = B * IT       # packed feature width (72)
FPAD = B * 32     # padded feature width (128)
U = 16            # units
IN_DIM = 2
HC = C // 2       # half own-node width (512)

C0 = 4096.0       # z0 = C0 * d_inv * x0     (fp8-range normalizer)
C1 = 262144.0     # z1 = C1 * d_inv * x1
S0 = 1.0 / C0
S1 = 2.0 / C1

_CACHE = {}


def _build():
    nc = bacc.Bacc("TRN2", target_bir_lowering=False, debug=False,
                   num_devices=NCORES)

    a_d = nc.dram_tensor("a", [P, KCH, C], FP8, kind="ExternalInput")
    z0_d = nc.dram_tensor("z0", [P, KCH, FPAD], FP8, kind="ExternalInput")
    x0T_d = nc.dram_tensor("x0T", [P, C], BF16, kind="ExternalInput")
    hxT_d = nc.dram_tensor("hxT", [P, C], BF16, kind="ExternalInput")
    wruR_d = nc.dram_tensor("wruR", [P, 3, 32], BF16, kind="ExternalInput")
    wruU_d = nc.dram_tensor("wruU", [P, 3, 32], BF16, kind="ExternalInput")
    wc_d = nc.dram_tensor("wc", [P, 3, 32], BF16, kind="ExternalInput")
    brur_d = nc.dram_tensor("brur", [P, 1], F32, kind="ExternalInput")
    bruu_d = nc.dram_tensor("bruu", [P, 1], F32, kind="ExternalInput")
    bc_d = nc.dram_tensor("bc", [P, 1], F32, kind="ExternalInput")
    cdzA_d = nc.dram_tensor("cdzA", [P, MB], BF16, kind="ExternalInput")
    cdzB_d = nc.dram_tensor("cdzB", [P, MB], BF16, kind="ExternalInput")
    ident_d = nc.dram_tensor("ident", [P, P], BF16, kind="ExternalInput")
    out_d = nc.dram_tensor("out", [P, C], F32, kind="ExternalOutput")

    with tile.TileContext(nc) as tc:
        with (
            tc.tile_pool(name="big", bufs=1) as big,
            tc.tile_pool(name="psmm", bufs=2, space="PSUM") as psmm,
            tc.tile_pool(name="pstp", bufs=2, space="PSUM") as pstp,
            tc.tile_pool(name="psg", bufs=4, space="PSUM") as psg,
            tc.tile_pool(name="dram", bufs=1, space="DRAM") as dram,
        ):
            # ---------- persistent SBUF tensors ----------
            NDMA = 16
            CPI = KCH // NDMA  # chunks per stream DMA (4)
            abf = [big.tile([P, CPI, C], FP8, name=f"abf{i}")
                   for i in range(NDMA)]

            z0 = big.tile([P, KCH, FPAD], FP8)
            zgA = big.tile([P, KCH, FPAD], FP8)    # gathered z1 / z1c
            zgB = big.tile([P, KCH, FPAD], FP8)    # gathered z0c
            x0T = big.tile([P, C], BF16)           # becomes x0cT after rh
            hxT = big.tile([P, C], BF16)
            y1raw = big.tile([P, C], BF16)         # c0*x1 own (raw psum)
            x2raw = big.tile([P, C], BF16)         # c1/2*T@x1 own
            y1craw = big.tile([P, C], BF16)
            x2craw = big.tile([P, C], BF16)
            sigR = big.tile([P, C], BF16)
            sigU = big.tile([P, C], BF16)
            cT = big.tile([P, C], BF16)
            outT = big.tile([P, C], F32)
            wruR = big.tile([P, 3, 32], BF16)
            wruU = big.tile([P, 3, 32], BF16)
            wc = big.tile([P, 3, 32], BF16)
            brur = big.tile([P, 1], F32)
            bruu = big.tile([P, 1], F32)
            bc = big.tile([P, 1], F32)
            cdzA = big.tile([P, MB], BF16)
            cdzB = big.tile([P, MB], BF16)
            identbf = big.tile([P, P], BF16)
            pkA = big.tile([P, MB, B, 32], FP8)    # AG payloads (padded,
            pkB = big.tile([P, MB, B, 32], FP8)    #  pre-scaled)
            pkC = big.tile([P, MB, B, 32], FP8)

            # ---------- input DMAs ----------
            nc.scalar.dma_start(z0[:], z0_d[:])
            nc.scalar.dma_start(x0T[:], x0T_d[:])
            nc.scalar.dma_start(hxT[:], hxT_d[:])
            nc.gpsimd.dma_start(wruR[:], wruR_d[:])
            nc.gpsimd.dma_start(wruU[:], wruU_d[:])
            nc.gpsimd.dma_start(wc[:], wc_d[:])
            nc.gpsimd.dma_start(brur[:], brur_d[:])
            nc.gpsimd.dma_start(bruu[:], bruu_d[:])
            nc.gpsimd.dma_start(bc[:], bc_d[:])
            nc.gpsimd.dma_start(cdzA[:], cdzA_d[:])
            nc.gpsimd.dma_start(cdzB[:], cdzB_d[:])
            nc.gpsimd.dma_start(identbf[:], ident_d[:])

            # ---------- adjacency stream (fp8, p-major contiguous) ----------
            # scalar carries z0/x0T/hxT (1.5 MB) so it gets fewer tiles;
            # queue order interleaves so chunks arrive roughly in kp order
            S, G, Csc = nc.sync, nc.gpsimd, nc.scalar
            ENGS = [S, G, Csc, S, G, S, G, Csc, S, G, S, Csc, G, S, G, Csc]
            for i in range(NDMA):
                ENGS[i].dma_start(abf[i][:], a_d[:, i * CPI:(i + 1) * CPI, :])

            def apair(kp, h):
                """Moving operand [128, 2, 512] for chunk pair kp, half h."""
                i, kk = (2 * kp) // CPI, (2 * kp) % CPI
                return abf[i][:, kk:kk + 2, h * HC:(h + 1) * HC]

            def diffusion(ps, z):
                """Both halves, one LDWEIGHTS per chunk pair (the weight
                load is the PE rate limiter; the two half-matmuls share
                the stationary operand)."""
                for kp in range(KP):
                    lz = z[:, 2 * kp:2 * kp + 2, :]
                    nc.tensor.ldweights(lz, perf_mode=DR)
                    for h in range(2):
                        mm = nc.tensor.matmul(
                            ps[h][:], lhsT=lz, rhs=apair(kp, h),
                            start=(kp == 0), stop=(kp == KP - 1),
                            perf_mode=DR,
                        )
                        mm.ins.ldweights = False

            def transposes(srcT, pk, cdz, mbs):
                """srcT node-block -> node-major padded, scaled by cdz."""
                for mb in mbs:
                    pt = pstp.tile([P, P], BF16, tag="tp")
                    nc.tensor.transpose(
                        pt[:], srcT[:, mb * P:(mb + 1) * P], identbf[:])
                    cdb = cdz[:, mb:mb + 1].unsqueeze(-1).broadcast_to(
                        (P, B, 32))
                    nc.vector.tensor_tensor(
                        pk[:, mb, :, :],
                        pt[:].rearrange("p (b e) -> p b e", b=B),
                        cdb, ALU.mult)

            def allgather(pk, zdst):
                agin = dram.tile([P, MB, B, 32], FP8, tag="agin")
                agout = dram.tile([NCORES, P, MB, B, 32], FP8,
                                  addr_space="Shared", tag="agout")
                # gpsimd: its stream share drains early, so the payload
                # DMA + trigger don't queue behind stream traffic
                nc.gpsimd.dma_start(agin[:], pk[:])
                nc.gpsimd.collective_compute(
                    "AllGather", ALU.bypass,
                    replica_groups=[list(range(NCORES))],
                    ins=[agin[:]], outs=[agout[:]],
                )
                # gather lands directly in the z tile, split across queues
                for eng, r0, r1 in ((nc.sync, 0, 3), (nc.scalar, 3, 6),
                                    (nc.gpsimd, 6, 8)):
                    eng.dma_start(
                        zdst[:, r0 * MB:r1 * MB, :].rearrange(
                            "p (r m) f -> p r (m f)", r=r1 - r0),
                        agout[r0:r1].rearrange("r p m b e -> p r (m b e)"),
                    )

            def gate_m01(wbf, srcs01, gname):
                """Open gate psum groups with the m=0,1 terms (early)."""
                halves = []
                for h in range(2):
                    pg = psg.tile([P, HC], F32, tag="gate",
                                  name=f"pg{gname}{h}", bufs=4)
                    for b in range(B):
                        for m in range(2):
                            nc.tensor.matmul(
                                pg[b * 32:(b + 1) * 32, :],
                                lhsT=wbf[b * 32:b * 32 + IT, m, :],
                                rhs=srcs01[m][b * 32:b * 32 + IT,
                                              h * HC:(h + 1) * HC],
                                start=(m == 0), stop=False,
                                tile_position=(b * 32, b * 32),
                            )
                    halves.append(pg)
                return halves

            def gate_m2(pg, wbf, src2, h):
                for b in range(B):
                    nc.tensor.matmul(
                        pg[b * 32:(b + 1) * 32, :],
                        lhsT=wbf[b * 32:b * 32 + IT, 2, :],
                        rhs=src2[b * 32:b * 32 + IT, h * HC:(h + 1) * HC],
                        start=False, stop=True,
                        tile_position=(b * 32, b * 32),
                    )

            # ================= gconv 1 (r/u gates) =================
            # diff A chases the stream
            psA = [psmm.tile([P, HC], F32, tag="mm", name=f"psA{h}")
                   for h in range(2)]
            diffusion(psA, z0)
            for h in range(2):
                nc.vector.tensor_copy(y1raw[:, h * HC:(h + 1) * HC],
                                      psA[h][:])
            transposes(y1raw, pkA, cdzA, range(MB))
            allgather(pkA, zgA)
            # r/u gate m=0,1 run inside the AG window
            pg_r = gate_m01(wruR, [x0T, y1raw], "r")
            pg_u = gate_m01(wruU, [x0T, y1raw], "u")

            # diff B, then per-half tail
            psB = [psmm.tile([P, HC], F32, tag="mm", name=f"psB{h}")
                   for h in range(2)]
            diffusion(psB, zgA)
            for h in range(2):
                fs = slice(h * HC, (h + 1) * HC)
                nc.vector.tensor_copy(x2raw[:, fs], psB[h][:])
                gate_m2(pg_r[h], wruR, x2raw, h)
                gate_m2(pg_u[h], wruU, x2raw, h)
                nc.scalar.activation(sigR[:, fs], pg_r[h][:], AF.Sigmoid,
                                     bias=brur[:])
                for b in range(B):
                    nc.vector.tensor_tensor(
                        x0T[b * 32:b * 32 + U, fs],
                        sigR[b * 32:b * 32 + U, fs],
                        hxT[b * 32:b * 32 + U, fs],
                        ALU.mult,
                    )
                transposes(x0T, pkB, cdzB, range(h * MB // 2,
                                                 (h + 1) * MB // 2))
            allgather(pkB, zgB)
            for h in range(2):
                nc.scalar.activation(sigU[:, h * HC:(h + 1) * HC],
                                     pg_u[h][:], AF.Sigmoid, bias=bruu[:])

            # ================= gconv 2 (candidate c) =================
            psC = [psmm.tile([P, HC], F32, tag="mm", name=f"psC{h}")
                   for h in range(2)]
            diffusion(psC, zgB)
            for h in range(2):
                fs = slice(h * HC, (h + 1) * HC)
                nc.vector.tensor_copy(y1craw[:, fs], psC[h][:])
                transposes(y1craw, pkC, cdzA, range(h * MB // 2,
                                                    (h + 1) * MB // 2))
            allgather(pkC, zgA)
            pg_c = gate_m01(wc, [x0T, y1craw], "c")

            # diff D + per-half tail to the output DMA
            psD = [psmm.tile([P, HC], F32, tag="mm", name=f"psD{h}")
                   for h in range(2)]
            diffusion(psD, zgA)
            for h in range(2):
                fs = slice(h * HC, (h + 1) * HC)
                nc.vector.tensor_copy(x2craw[:, fs], psD[h][:])
                gate_m2(pg_c[h], wc, x2craw, h)
                nc.scalar.activation(cT[:, fs], pg_c[h][:], AF.Tanh,
                                     bias=bc[:])
                # out = c + u*(h - c)
                eng = nc.gpsimd if h == 0 else nc.vector
                eng.tensor_tensor(outT[:, fs], hxT[:, fs], cT[:, fs],
                                  ALU.subtract)
                eng.tensor_tensor(outT[:, fs], outT[:, fs], sigU[:, fs],
                                  ALU.mult)
                eng.tensor_tensor(outT[:, fs], outT[:, fs], cT[:, fs],
                                  ALU.add)
                (nc.sync if h == 0 else nc.scalar).dma_start(
                    out_d[:, fs], outT[:, fs])

    nc.compile()
    return nc


def _get_nc():
    if "nc" not in _CACHE:
        _CACHE["nc"] = _build()
    return _CACHE["nc"]


# feature permutation: device feature j -> reference feature i
# j = 0..15 -> i = j+2 (hidden), j = 16,17 -> i = j-16 (input x)
_PERM = np.array(list(range(2, 18)) + [0, 1])


def _host_prep(inputs, hx, adj, W_ru, b_ru, W_c, b_c):
    f32 = np.float32
    bf16 = ml_dtypes.bfloat16
    fp8 = ml_dtypes.float8_e4m3fn

    xr = np.ascontiguousarray(inputs, dtype=f32).reshape(B, N, IN_DIM)
    hr = np.ascontiguousarray(hx, dtype=f32).reshape(B, N, U)
    x0 = np.concatenate([hr, xr], axis=2).transpose(1, 0, 2)  # [N, B, 18]
    x0 = np.ascontiguousarray(x0).reshape(N, FP)

    adj = np.asarray(adj, f32)
    d = adj.sum(axis=1) + 1.0
    dinv = 1.0 / d

    # z0 = C0 * dinv * x0, padded to 32 cols per batch, p-major, fp8
    z0 = np.zeros((N, B, 32), f32)
    z0[:, :, 0:IT] = (C0 * dinv)[:, None, None] * x0.reshape(N, B, IT)
    z0 = z0.reshape(KCH, P, FPAD).transpose(1, 0, 2)  # [P, KCH, FPAD]
    z0 = np.ascontiguousarray(z0).astype(fp8)

    def pad_w(w, lo):
        # fold the diffusion unscales into the weights:
        # gate = x0*(W0-W2) + (c0*x1)*(W1/c0) + (c1/2*Tx1)*(2*W2/c1)
        w3 = np.asarray(w, f32).reshape(IT, 3, -1)[_PERM][:, :, lo:lo + U]
        w3 = np.stack([w3[:, 0] - w3[:, 2], S0 * w3[:, 1], S1 * w3[:, 2]],
                      axis=1)
        out = np.zeros((B, 32, 3, 32), f32)
        out[:, 0:IT, :, 0:U] = w3[None]
        return out.reshape(P, 3, 32).astype(bf16)

    wruR_p = pad_w(W_ru, 0)
    wruU_p = pad_w(W_ru, U)
    wc_p = pad_w(W_c, 0)

    def pad_bias(v):
        t = np.zeros((B, 32), f32)
        t[:, 0:U] = np.asarray(v, f32)
        return np.ascontiguousarray(t.reshape(P)[:, None])

    brur_t = pad_bias(np.asarray(b_ru, f32)[0:U])
    bruu_t = pad_bias(np.asarray(b_ru, f32)[U:2 * U])
    bc_t = pad_bias(np.asarray(b_c, f32))
    ident = np.eye(P, dtype=f32).astype(bf16)

    in_maps = []
    for m in range(NCORES):
        sl = slice(m * C, (m + 1) * C)
        a_m = np.ascontiguousarray(adj[:, sl])
        a_m[m * C + np.arange(C), np.arange(C)] += 1.0
        # p-major: [p, k, c] = row k*128+p
        a_m = a_m.reshape(KCH, P, C).transpose(1, 0, 2)
        a_m = np.ascontiguousarray(a_m).astype(fp8)

        x0own = x0[sl]
        x0T = np.zeros((B, 32, C), f32)
        x0T[:, 0:IT, :] = x0own.reshape(C, B, IT).transpose(1, 2, 0)
        x0T = x0T.reshape(P, C).astype(bf16)
        hxT_p = np.zeros((B, 32, C), f32)
        hxT_p[:, 0:U, :] = hr[:, sl, :].transpose(0, 2, 1)
        hxT_p = hxT_p.reshape(P, C).astype(bf16)
        # per-own-node payload scales, node-major [p, mb]
        dlocal = dinv[sl].reshape(MB, P).T
        cdzA_p = np.ascontiguousarray((C1 / C0) * dlocal).astype(bf16)
        cdzB_p = np.ascontiguousarray(C0 * dlocal).astype(bf16)
        in_maps.append({
            "a": a_m,
            "z0": z0,
            "x0T": x0T,
            "hxT": hxT_p,
            "wruR": wruR_p,
            "wruU": wruU_p,
            "wc": wc_p,
            "brur": brur_t,
            "bruu": bruu_t,
            "bc": bc_t,
            "cdzA": cdzA_p,
            "cdzB": cdzB_p,
            "ident": ident,
        })
    return in_maps


def _run(in_maps, trace=False, **kw):
    nc = _get_nc()
    return bass_utils.run_bass_kernel_spmd(
        nc, in_maps, core_ids=list(range(NCORES)), trace=trace, **kw)


def _assemble(results):
    out = np.empty((B, N * U), np.float32)
    for m in range(NCORES):
        # device layout [b*32+u, n] (rows 16..31 per block are padding)
        blk = results[m]["out"].reshape(B, 32, C)[:, 0:U, :].transpose(0, 2, 1)
        out[:, m * C * U:(m + 1) * C * U] = blk.reshape(B, C * U)
    return out


def kernel(inputs, hx, adj, W_ru, b_ru, W_c, b_c):
    in_maps = _host_prep(inputs, hx, adj, W_ru, b_ru, W_c, b_c)
    res = _run(in_maps)
    return _assemble(res.results)
